# revision 37
# baseline (speedup 1.0000x reference)
"""CARAFE on 8 Trainium2 NeuronCores — v5: 25-tap packed contraction.

out[n,c,2h+a,2w+b] = sum_{i,j in 5x5} f[n,c,h+i-2,w+j-2] * m[n,5i+j,2h+a,2w+b]

Per core = one (n, h-half) shard, 32 low-res rows ("jobs").  W is split
into 4 tiles of 16; for each (job, w-tile) ONE bf16 matmul contracts all
25 taps at once: contraction = (i, w'') = 5 kernel rows x 20 padded
feature cols = 100 partitions; moving = banded masks [100, 64] (cols =
4w+2b+a, fully contiguous); stationary = replicated features
ftr[(i,w''), (hl,t,c)] = f[hl+i-2, 16t+w''-2] (host-built, 3.3 MB).
psum col = 4*w_out+2b+a; the (w,b,a)->(a,wup) permute happens in the
PSUM->SBUF copy.  128 matmuls of 64 cols replace 160 of 256 cols,
making the kernel DMA-bound, so the schedule is built around the input
stream: the mask bands are host-densified (diagonal placement + zeros
baked in, 64 cols per job-tile) so every transfer is a plain contiguous
2D DMA; ftr and mask chunks interleave in consumption order across the
two HWDGE queues while outputs (bf16, upcast on host; tol 2e-2) ride
gpsimd's software DGE queue.
"""
import sys

if "/opt/trn_rl_repo" not in sys.path:
    sys.path.insert(0, "/opt/trn_rl_repo")

from contextlib import ExitStack

import numpy as np
import ml_dtypes

import concourse.tile as tile
from concourse import bacc, mybir
from concourse.ap import AP
from concourse.bass_utils import run_bass_kernel_spmd

# ---- problem constants (hardcoded per harness contract) ----
N = 4
C = 128
H = 64
W = 64
KS = 5
PAD = 2
SCALE = 2
WP = W + KS - 1          # 68 padded feature cols
NB = SCALE * W           # 128 upsampled cols per hup row
NH = H // 2              # 32 low-res rows per core
NROWS = NH + 4           # 36 feature rows per shard (halo zero-padded)
TP = 16                  # w-tile width
NT = W // TP             # 4 w-tiles
TPP = TP + KS - 1        # 20 padded cols per tile -> contraction 5*20=100
CONTR = KS * TPP         # 100
SUB = 4 * KS             # 20 band elems per (partition, job, tile)
BWT = 4 * TP             # 64 band cols per (job, tile); edge runs clipped
JOBW = NT * BWT          # 256 band cols per job
N_BBUF = 16              # job band buffers across the rotation
TILE_SZ = (4, 4, 4, 4)
TILE_LO = (0, 4, 8, 12)
OBATCH = 4               # jobs per output DMA

F32 = mybir.dt.float32
BF16 = mybir.dt.bfloat16

_PROG_CACHE: dict = {}


def _device_body(tc, ctx, out_ap, ftr_ap, msk_ap):
    nc = tc.nc
    sb = ctx.enter_context(tc.tile_pool(name="sb", bufs=1))
    psum = ctx.enter_context(tc.tile_pool(name="ps", bufs=6, space="PSUM"))
    obp = ctx.enter_context(tc.tile_pool(name="ob", bufs=3))

    ftr = sb.tile([CONTR, NH * NT * C], BF16)
    N_TILE = len(TILE_SZ)
    tiles = [
        sb.tile([CONTR, TILE_SZ[t] * JOBW], BF16, name=f"band{t}", tag=f"band{t}")
        for t in range(N_TILE)
    ]

    # ftr is fully resident: 8 4-job chunks, all upfront, both HWDGE queues
    def ftr_chunk(k, eng):
        lo, hi = k * 4 * NT * C, (k + 1) * 4 * NT * C
        eng.dma_start(ftr[:, lo:hi], ftr_ap[:, lo:hi])

    # band tiles are host-densified (diagonal placement + zeros baked into
    # the mskp image), so a (re)fill is a plain contiguous 2D DMA
    def scatter(t, job_lo, eng):
        nj = TILE_SZ[t]
        eng.dma_start(
            tiles[t][:], msk_ap[:, job_lo * JOBW : (job_lo + nj) * JOBW]
        )

    # consumption-ordered interleave: job 4k needs ftr chunk k + mask tile k.
    # Latency-critical first-half inputs ride the two HWDGE queues; the
    # latency-tolerant second-half ftr chunks ride gpsimd's software queue.
    ftr_chunk(0, nc.sync)
    scatter(0, 0, nc.scalar)
    scatter(1, 4, nc.sync)
    ftr_chunk(1, nc.scalar)
    ftr_chunk(2, nc.sync)
    scatter(2, 8, nc.scalar)
    scatter(3, 12, nc.sync)
    ftr_chunk(3, nc.scalar)
    ftr_chunk(4, nc.sync)
    ftr_chunk(5, nc.scalar)
    ftr_chunk(6, nc.sync)
    ftr_chunk(7, nc.scalar)

    def tile_of(hl):
        b16 = hl % N_BBUF
        for t in range(N_TILE - 1, -1, -1):
            if b16 >= TILE_LO[t]:
                return t, b16 - TILE_LO[t]
        raise AssertionError

    ob = None
    for hl in range(NH):
        t, buf = tile_of(hl)
        tap = tiles[t][:]
        TW = TILE_SZ[t] * JOBW
        ps = psum.tile([C, 4 * TP * NT], F32)
        psap = ps[:]
        for wt in range(NT):
            lhsT = ftr[:, (hl * NT + wt) * C : (hl * NT + wt + 1) * C]
            rhs = AP(
                tap.tensor,
                tap.offset + buf * JOBW + wt * BWT,
                [[TW, CONTR], [1, 4 * TP]],
            )
            nc.tensor.matmul(
                ps[:, wt * 4 * TP : (wt + 1) * 4 * TP], lhsT, rhs,
                start=True, stop=True,
            )

        if hl % OBATCH == 0:
            ob = obp.tile([C, OBATCH * 2 * NB], BF16)
        sl = ob[:, (hl % OBATCH) * 2 * NB : (hl % OBATCH + 1) * 2 * NB]
        # permute psum (w_out, b, a) -> output (a, wup=2w_out+b) in the copy
        src = AP(psap.tensor, psap.offset, [[2 * NB, C], [1, 2], [4, W], [2, 2]])
        if hl % 4 == 3:
            nc.scalar.copy(sl, src)
        else:
            nc.vector.tensor_copy(sl, src)
        # last reader of this tile in the rotation round -> refill it
        if buf == TILE_SZ[t] - 1 and hl + N_BBUF - TILE_SZ[t] + 1 < NH:
            job_lo = hl + N_BBUF - TILE_SZ[t] + 1
            scatter(t, job_lo, nc.sync if t % 2 == 0 else nc.scalar)
        if hl % OBATCH == OBATCH - 1:
            # outputs ride gpsimd's software DGE queue, keeping both HWDGE
            # queues free for the input stream
            g0 = hl - (OBATCH - 1)
            nc.gpsimd.dma_start(out_ap[:, 2 * g0 : 2 * g0 + 2 * OBATCH, :], ob[:])


def _build_program():
    nc = bacc.Bacc(
        "TRN2", debug=False, enable_asserts=False, target_bir_lowering=False
    )
    ftr_t = nc.dram_tensor("ftr", [CONTR, NH * NT * C], BF16, kind="ExternalInput")
    msk_t = nc.dram_tensor("mskp", [CONTR, NH * JOBW], BF16, kind="ExternalInput")
    out_t = nc.dram_tensor("out", [C, 2 * NH, NB], BF16, kind="ExternalOutput")

    with tile.TileContext(nc) as tc, ExitStack() as ctx:
        _device_body(tc, ctx, out_t.ap(), ftr_t.ap(), msk_t.ap())
    nc.compile()
    return nc


def _prep_ftr(feat_n: np.ndarray, h0: int) -> np.ndarray:
    """[C,H,W] -> ftr[(i,w''), (hl, t, c)] bf16 [100, NH*4*C]:
    ftr[i*20+w'', hl, t, c] = f[c, h0+hl+i-2, 16t+w''-2] (zero-padded)."""
    fT = np.zeros((WP, NROWS, C), ml_dtypes.bfloat16)
    r_lo, r_hi = h0 - 2, h0 + NH + 2
    s_lo, s_hi = max(r_lo, 0), min(r_hi, H)
    fT[PAD : PAD + W, s_lo - r_lo : s_hi - r_lo, :] = (
        feat_n[:, s_lo:s_hi, :].transpose(2, 1, 0).astype(ml_dtypes.bfloat16)
    )
    ftr = np.empty((KS, TPP, NH, NT, C), ml_dtypes.bfloat16)
    for i in range(KS):
        for t in range(NT):
            ftr[i, :, :, t, :] = fT[TP * t : TP * t + TPP, i : i + NH, :]
    return np.ascontiguousarray(ftr.reshape(CONTR, NH * NT * C))


def _prep_mskp(masks_n: np.ndarray, h0: int) -> np.ndarray:
    """[25, 2H, 2W] -> dense band image mskp[(i,w''), (hl, t, col96)] bf16
    [100, NH*384]: run value masks[5i + (4-dw), 2(h0+hl)+a,
    clip(2(16t + w''-4+dw)+b)] at col 4*w'' + (4dw+2b+a); zeros elsewhere.
    """
    t20 = np.arange(SUB)
    dw = t20 // 4
    b = (t20 % 4) // 2
    a = t20 % 2
    i_ar = np.arange(KS).reshape(KS, 1, 1, 1, 1)
    w2 = np.arange(TPP).reshape(1, TPP, 1, 1, 1)
    hl = np.arange(NH).reshape(1, 1, NH, 1, 1)
    tt = np.arange(NT).reshape(1, 1, 1, NT, 1)
    k = 5 * i_ar + (4 - dw)                                  # [5,1,1,1,20]
    hup = 2 * (h0 + hl) + a                                  # [1,1,NH,1,20]
    wup = np.clip(2 * (TP * tt + w2 - 4 + dw) + b, 0, 2 * W - 1)
    vals = masks_n[k, hup, wup].astype(ml_dtypes.bfloat16)   # [5,TPP,NH,NT,20]
    vals = vals.reshape(KS, TPP, NH, NT, KS, 4)              # (.., dw, (b,a))
    band = np.zeros((KS, TPP, NH, NT, BWT), ml_dtypes.bfloat16)
    for w2 in range(TPP):
        for dw in range(KS):
            w = w2 - 4 + dw
            if 0 <= w < TP:
                band[:, w2, :, :, 4 * w : 4 * w + 4] = vals[:, w2, :, :, dw]
    return np.ascontiguousarray(band.reshape(CONTR, NH * JOBW))


def kernel(features: np.ndarray, masks: np.ndarray, _perf: dict | None = None):
    features = np.asarray(features, dtype=np.float32)
    masks = np.asarray(masks, dtype=np.float32)

    if "nc" not in _PROG_CACHE:
        _PROG_CACHE["nc"] = _build_program()
    nc = _PROG_CACHE["nc"]

    in_maps = []
    for core in range(8):
        n, half = divmod(core, 2)
        h0 = NH * half
        in_maps.append(
            {
                "ftr": _prep_ftr(features[n], h0),
                "mskp": _prep_mskp(masks[n], h0),
            }
        )

    trace = bool(_perf is not None and _perf.get("trace"))
    res = run_bass_kernel_spmd(
        nc, in_maps, core_ids=list(range(8)), trace=trace,
        **({} if not trace else {"trace_cores": [0]}),
    )
    if _perf is not None:
        _perf["exec_time_ns"] = res.exec_time_ns
        _perf["trace"] = res.instructions_and_trace

    out = np.empty((N, C, SCALE * H, SCALE * W), np.float32)
    for core in range(8):
        n, half = divmod(core, 2)
        out[n, :, 64 * half : 64 * half + 64, :] = res.results[core]["out"].astype(
            np.float32
        )
    return out


# revision 40
# speedup vs baseline: 1.0175x; 1.0175x over previous
"""CARAFE on 8 Trainium2 NeuronCores — v5: 25-tap packed contraction.

out[n,c,2h+a,2w+b] = sum_{i,j in 5x5} f[n,c,h+i-2,w+j-2] * m[n,5i+j,2h+a,2w+b]

Per core = one (n, h-half) shard, 32 low-res rows ("jobs").  W is split
into 4 tiles of 16; for each (job, w-tile) ONE bf16 matmul contracts all
25 taps at once: contraction = (i, w'') = 5 kernel rows x 20 padded
feature cols = 100 partitions; moving = banded masks [100, 64] (cols =
4w+2b+a, fully contiguous); stationary = replicated features
ftr[(i,w''), (hl,t,c)] = f[hl+i-2, 16t+w''-2] (host-built, 3.3 MB).
psum col = 4*w_out+2b+a; the (w,b,a)->(a,wup) permute happens in the
PSUM->SBUF copy.  128 matmuls of 64 cols replace 160 of 256 cols,
making the kernel DMA-bound, so the schedule is built around the input
stream: the mask bands are host-densified (diagonal placement + zeros
baked in, 64 cols per job-tile) so every transfer is a plain contiguous
2D DMA; ftr and mask chunks interleave in consumption order across the
two HWDGE queues while outputs (bf16, upcast on host; tol 2e-2) ride
gpsimd's software DGE queue.
"""
import sys

if "/opt/trn_rl_repo" not in sys.path:
    sys.path.insert(0, "/opt/trn_rl_repo")

from contextlib import ExitStack

import numpy as np
import ml_dtypes

import concourse.tile as tile
from concourse import bacc, mybir
from concourse.ap import AP
from concourse.bass_utils import run_bass_kernel_spmd

# ---- problem constants (hardcoded per harness contract) ----
N = 4
C = 128
H = 64
W = 64
KS = 5
PAD = 2
SCALE = 2
WP = W + KS - 1          # 68 padded feature cols
NB = SCALE * W           # 128 upsampled cols per hup row
NH = H // 2              # 32 low-res rows per core
NROWS = NH + 4           # 36 feature rows per shard (halo zero-padded)
TP = 16                  # w-tile width
NT = W // TP             # 4 w-tiles
TPP = TP + KS - 1        # 20 padded cols per tile -> contraction 5*20=100
CONTR = KS * TPP         # 100
SUB = 4 * KS             # 20 band elems per (partition, job, tile)
BWT = 4 * TP             # 64 band cols per (job, tile); edge runs clipped
JOBW = NT * BWT          # 256 band cols per job
N_BBUF = 16              # job band buffers across the rotation
TILE_SZ = (4, 4, 4, 4)
TILE_LO = (0, 4, 8, 12)
OBATCH = 4               # jobs per output DMA

F32 = mybir.dt.float32
BF16 = mybir.dt.bfloat16

_PROG_CACHE: dict = {}


def _device_body(tc, ctx, out_ap, ftr_ap, msk_ap):
    nc = tc.nc
    sb = ctx.enter_context(tc.tile_pool(name="sb", bufs=1))
    psum = ctx.enter_context(tc.tile_pool(name="ps", bufs=6, space="PSUM"))
    obp = ctx.enter_context(tc.tile_pool(name="ob", bufs=3))

    ftr = sb.tile([CONTR, NH * NT * C], BF16)
    N_TILE = len(TILE_SZ)
    tiles = [
        sb.tile([CONTR, TILE_SZ[t] * JOBW], BF16, name=f"band{t}", tag=f"band{t}")
        for t in range(N_TILE)
    ]

    # ftr is fully resident; chunks sized/ordered to match the consumption
    # wave (job j needs ftr jobs [j, j+1) and its mask tile together)
    def ftr_chunk(jlo, jhi, eng):
        lo, hi = jlo * NT * C, jhi * NT * C
        eng.dma_start(ftr[:, lo:hi], ftr_ap[:, lo:hi])

    # band tiles are host-densified (diagonal placement + zeros baked into
    # the mskp image), so a (re)fill is a plain contiguous 2D DMA
    def scatter(t, job_lo, eng):
        nj = TILE_SZ[t]
        eng.dma_start(
            tiles[t][:], msk_ap[:, job_lo * JOBW : (job_lo + nj) * JOBW]
        )

    # consumption-ordered interleave: job 4k needs ftr chunk k + mask tile k.
    # Latency-critical first-half inputs ride the two HWDGE queues; the
    # latency-tolerant second-half ftr chunks ride gpsimd's software queue.
    ftr_chunk(0, 2, nc.sync)       # jobs 0-1 first: smallest possible gate
    scatter(0, 0, nc.scalar)       # masks jobs 0-3
    ftr_chunk(2, 4, nc.sync)
    scatter(1, 4, nc.sync)
    ftr_chunk(4, 8, nc.scalar)
    ftr_chunk(8, 12, nc.sync)
    scatter(2, 8, nc.scalar)
    scatter(3, 12, nc.sync)
    ftr_chunk(12, 16, nc.scalar)

    def tile_of(hl):
        b16 = hl % N_BBUF
        for t in range(N_TILE - 1, -1, -1):
            if b16 >= TILE_LO[t]:
                return t, b16 - TILE_LO[t]
        raise AssertionError

    ob = None
    for hl in range(NH):
        t, buf = tile_of(hl)
        tap = tiles[t][:]
        TW = TILE_SZ[t] * JOBW
        ps = psum.tile([C, 4 * TP * NT], F32)
        psap = ps[:]
        for wt in range(NT):
            lhsT = ftr[:, (hl * NT + wt) * C : (hl * NT + wt + 1) * C]
            rhs = AP(
                tap.tensor,
                tap.offset + buf * JOBW + wt * BWT,
                [[TW, CONTR], [1, 4 * TP]],
            )
            nc.tensor.matmul(
                ps[:, wt * 4 * TP : (wt + 1) * 4 * TP], lhsT, rhs,
                start=True, stop=True,
            )

        if hl % OBATCH == 0:
            ob = obp.tile([C, OBATCH * 2 * NB], BF16)
        sl = ob[:, (hl % OBATCH) * 2 * NB : (hl % OBATCH + 1) * 2 * NB]
        # permute psum (w_out, b, a) -> output (a, wup=2w_out+b) in the copy
        src = AP(psap.tensor, psap.offset, [[2 * NB, C], [1, 2], [4, W], [2, 2]])
        if hl % 4 == 3:
            nc.scalar.copy(sl, src)
        else:
            nc.vector.tensor_copy(sl, src)
        # last reader of this tile in the rotation round -> refill it, and
        # bring in the matching second-half ftr jobs on the other queue so
        # queue FIFO order equals consumption order
        if buf == TILE_SZ[t] - 1 and hl + N_BBUF - TILE_SZ[t] + 1 < NH:
            job_lo = hl + N_BBUF - TILE_SZ[t] + 1
            e1 = nc.sync if t % 2 == 0 else nc.scalar
            e2 = nc.scalar if t % 2 == 0 else nc.sync
            scatter(t, job_lo, e1)
            ftr_chunk(job_lo, job_lo + TILE_SZ[t], e2)
        if hl % OBATCH == OBATCH - 1:
            # outputs ride gpsimd's software DGE queue, keeping both HWDGE
            # queues free for the input stream
            g0 = hl - (OBATCH - 1)
            nc.gpsimd.dma_start(out_ap[:, 2 * g0 : 2 * g0 + 2 * OBATCH, :], ob[:])


def _build_program():
    nc = bacc.Bacc(
        "TRN2", debug=False, enable_asserts=False, target_bir_lowering=False
    )
    ftr_t = nc.dram_tensor("ftr", [CONTR, NH * NT * C], BF16, kind="ExternalInput")
    msk_t = nc.dram_tensor("mskp", [CONTR, NH * JOBW], BF16, kind="ExternalInput")
    out_t = nc.dram_tensor("out", [C, 2 * NH, NB], BF16, kind="ExternalOutput")

    with tile.TileContext(nc) as tc, ExitStack() as ctx:
        _device_body(tc, ctx, out_t.ap(), ftr_t.ap(), msk_t.ap())
    nc.compile()
    return nc


def _prep_ftr(feat_n: np.ndarray, h0: int) -> np.ndarray:
    """[C,H,W] -> ftr[(i,w''), (hl, t, c)] bf16 [100, NH*4*C]:
    ftr[i*20+w'', hl, t, c] = f[c, h0+hl+i-2, 16t+w''-2] (zero-padded)."""
    fT = np.zeros((WP, NROWS, C), ml_dtypes.bfloat16)
    r_lo, r_hi = h0 - 2, h0 + NH + 2
    s_lo, s_hi = max(r_lo, 0), min(r_hi, H)
    fT[PAD : PAD + W, s_lo - r_lo : s_hi - r_lo, :] = (
        feat_n[:, s_lo:s_hi, :].transpose(2, 1, 0).astype(ml_dtypes.bfloat16)
    )
    ftr = np.empty((KS, TPP, NH, NT, C), ml_dtypes.bfloat16)
    for i in range(KS):
        for t in range(NT):
            ftr[i, :, :, t, :] = fT[TP * t : TP * t + TPP, i : i + NH, :]
    return np.ascontiguousarray(ftr.reshape(CONTR, NH * NT * C))


def _prep_mskp(masks_n: np.ndarray, h0: int) -> np.ndarray:
    """[25, 2H, 2W] -> dense band image mskp[(i,w''), (hl, t, col96)] bf16
    [100, NH*384]: run value masks[5i + (4-dw), 2(h0+hl)+a,
    clip(2(16t + w''-4+dw)+b)] at col 4*w'' + (4dw+2b+a); zeros elsewhere.
    """
    t20 = np.arange(SUB)
    dw = t20 // 4
    b = (t20 % 4) // 2
    a = t20 % 2
    i_ar = np.arange(KS).reshape(KS, 1, 1, 1, 1)
    w2 = np.arange(TPP).reshape(1, TPP, 1, 1, 1)
    hl = np.arange(NH).reshape(1, 1, NH, 1, 1)
    tt = np.arange(NT).reshape(1, 1, 1, NT, 1)
    k = 5 * i_ar + (4 - dw)                                  # [5,1,1,1,20]
    hup = 2 * (h0 + hl) + a                                  # [1,1,NH,1,20]
    wup = np.clip(2 * (TP * tt + w2 - 4 + dw) + b, 0, 2 * W - 1)
    vals = masks_n[k, hup, wup].astype(ml_dtypes.bfloat16)   # [5,TPP,NH,NT,20]
    vals = vals.reshape(KS, TPP, NH, NT, KS, 4)              # (.., dw, (b,a))
    band = np.zeros((KS, TPP, NH, NT, BWT), ml_dtypes.bfloat16)
    for w2 in range(TPP):
        for dw in range(KS):
            w = w2 - 4 + dw
            if 0 <= w < TP:
                band[:, w2, :, :, 4 * w : 4 * w + 4] = vals[:, w2, :, :, dw]
    return np.ascontiguousarray(band.reshape(CONTR, NH * JOBW))


def kernel(features: np.ndarray, masks: np.ndarray, _perf: dict | None = None):
    features = np.asarray(features, dtype=np.float32)
    masks = np.asarray(masks, dtype=np.float32)

    if "nc" not in _PROG_CACHE:
        _PROG_CACHE["nc"] = _build_program()
    nc = _PROG_CACHE["nc"]

    in_maps = []
    for core in range(8):
        n, half = divmod(core, 2)
        h0 = NH * half
        in_maps.append(
            {
                "ftr": _prep_ftr(features[n], h0),
                "mskp": _prep_mskp(masks[n], h0),
            }
        )

    trace = bool(_perf is not None and _perf.get("trace"))
    res = run_bass_kernel_spmd(
        nc, in_maps, core_ids=list(range(8)), trace=trace,
        **({} if not trace else {"trace_cores": [0]}),
    )
    if _perf is not None:
        _perf["exec_time_ns"] = res.exec_time_ns
        _perf["trace"] = res.instructions_and_trace

    out = np.empty((N, C, SCALE * H, SCALE * W), np.float32)
    for core in range(8):
        n, half = divmod(core, 2)
        out[n, :, 64 * half : 64 * half + 64, :] = res.results[core]["out"].astype(
            np.float32
        )
    return out


# revision 42
# speedup vs baseline: 1.1033x; 1.0844x over previous
"""CARAFE on 8 Trainium2 NeuronCores — v5: 25-tap packed contraction.

out[n,c,2h+a,2w+b] = sum_{i,j in 5x5} f[n,c,h+i-2,w+j-2] * m[n,5i+j,2h+a,2w+b]

Per core = one (n, h-half) shard, 32 low-res rows ("jobs").  W is split
into 4 tiles of 16; for each (job, w-tile) ONE bf16 matmul contracts all
25 taps at once: contraction = (i, w'') = 5 kernel rows x 20 padded
feature cols = 100 partitions; moving = banded masks [100, 64] (cols =
4w+2b+a, fully contiguous); stationary = replicated features
ftr[(i,w''), (hl,t,c)] = f[hl+i-2, 16t+w''-2] (host-built, 3.3 MB).
psum col = 4*w_out+2b+a; the (w,b,a)->(a,wup) permute happens in the
PSUM->SBUF copy.  128 matmuls of 64 cols replace 160 of 256 cols,
making the kernel DMA-bound, so the schedule is built around the input
stream: the mask bands are host-densified (diagonal placement + zeros
baked in, 64 cols per job-tile) so every transfer is a plain contiguous
2D DMA; ftr and mask chunks interleave in consumption order across the
two HWDGE queues while outputs (bf16, upcast on host; tol 2e-2) ride
gpsimd's software DGE queue.
"""
import sys

if "/opt/trn_rl_repo" not in sys.path:
    sys.path.insert(0, "/opt/trn_rl_repo")

from contextlib import ExitStack

import numpy as np
import ml_dtypes

import concourse.tile as tile
from concourse import bacc, mybir
from concourse.ap import AP
from concourse.bass_utils import run_bass_kernel_spmd

# ---- problem constants (hardcoded per harness contract) ----
N = 4
C = 128
H = 64
W = 64
KS = 5
PAD = 2
SCALE = 2
WP = W + KS - 1          # 68 padded feature cols
NB = SCALE * W           # 128 upsampled cols per hup row
NH = H // 2              # 32 low-res rows per core
NROWS = NH + 4           # 36 feature rows per shard (halo zero-padded)
TP = 16                  # w-tile width
NT = W // TP             # 4 w-tiles
TPP = TP + KS - 1        # 20 padded cols per tile -> contraction 5*20=100
CONTR = KS * TPP         # 100
SUB = 4 * KS             # 20 band elems per (partition, job, tile)
BWT = 4 * TP             # 64 band cols per (job, tile); edge runs clipped
JOBW = NT * BWT          # 256 band cols per job
N_BBUF = 16              # job band buffers across the rotation
TILE_SZ = (4, 4, 4, 4)
TILE_LO = (0, 4, 8, 12)
OBATCH = 4               # jobs per output DMA

F32 = mybir.dt.float32
BF16 = mybir.dt.bfloat16

_PROG_CACHE: dict = {}


def _device_body(tc, ctx, out_ap, ftr_ap, msk_ap):
    nc = tc.nc
    sb = ctx.enter_context(tc.tile_pool(name="sb", bufs=1))
    psum = ctx.enter_context(tc.tile_pool(name="ps", bufs=6, space="PSUM"))
    obp = ctx.enter_context(tc.tile_pool(name="ob", bufs=3))

    ftr = sb.tile([CONTR, NH * NT * C], BF16)
    N_TILE = len(TILE_SZ)
    tiles = [
        sb.tile([CONTR, TILE_SZ[t] * JOBW], BF16, name=f"band{t}", tag=f"band{t}")
        for t in range(N_TILE)
    ]

    # ftr is fully resident; chunks sized/ordered to match the consumption
    # wave (job j needs ftr jobs [j, j+1) and its mask tile together)
    def ftr_chunk(jlo, jhi, eng):
        lo, hi = jlo * NT * C, jhi * NT * C
        eng.dma_start(ftr[:, lo:hi], ftr_ap[:, lo:hi])

    # band tiles are host-densified (diagonal placement + zeros baked into
    # the mskp image), so a (re)fill is a plain contiguous 2D DMA
    def scatter(t, job_lo, eng):
        nj = TILE_SZ[t]
        eng.dma_start(
            tiles[t][:], msk_ap[:, job_lo * JOBW : (job_lo + nj) * JOBW]
        )

    # consumption-ordered interleave: job 4k needs ftr chunk k + mask tile k.
    # Latency-critical first-half inputs ride the two HWDGE queues; the
    # latency-tolerant second-half ftr chunks ride gpsimd's software queue.
    ftr_chunk(0, 2, nc.sync)       # jobs 0-1 first: smallest possible gate
    scatter(0, 0, nc.scalar)       # masks jobs 0-3
    ftr_chunk(2, 4, nc.sync)
    scatter(1, 4, nc.sync)
    ftr_chunk(4, 8, nc.scalar)
    ftr_chunk(8, 12, nc.sync)
    scatter(2, 8, nc.scalar)
    scatter(3, 12, nc.sync)
    ftr_chunk(12, 16, nc.scalar)
    # gpsimd's software queue is idle until the first output batch (~19us);
    # park the jobs-16-23 ftr chunks there to offload the HWDGE in-stream
    ftr_chunk(16, 20, nc.gpsimd)
    ftr_chunk(20, 24, nc.gpsimd)

    def tile_of(hl):
        b16 = hl % N_BBUF
        for t in range(N_TILE - 1, -1, -1):
            if b16 >= TILE_LO[t]:
                return t, b16 - TILE_LO[t]
        raise AssertionError

    ob = None
    for hl in range(NH):
        t, buf = tile_of(hl)
        tap = tiles[t][:]
        TW = TILE_SZ[t] * JOBW
        ps = psum.tile([C, 4 * TP * NT], F32)
        psap = ps[:]
        for wt in range(NT):
            lhsT = ftr[:, (hl * NT + wt) * C : (hl * NT + wt + 1) * C]
            rhs = AP(
                tap.tensor,
                tap.offset + buf * JOBW + wt * BWT,
                [[TW, CONTR], [1, 4 * TP]],
            )
            nc.tensor.matmul(
                ps[:, wt * 4 * TP : (wt + 1) * 4 * TP], lhsT, rhs,
                start=True, stop=True,
            )

        if hl % OBATCH == 0:
            ob = obp.tile([C, OBATCH * 2 * NB], BF16)
        sl = ob[:, (hl % OBATCH) * 2 * NB : (hl % OBATCH + 1) * 2 * NB]
        # permute psum (w_out, b, a) -> output (a, wup=2w_out+b) in the copy
        src = AP(psap.tensor, psap.offset, [[2 * NB, C], [1, 2], [4, W], [2, 2]])
        if hl % 4 == 3:
            nc.scalar.copy(sl, src)
        else:
            nc.vector.tensor_copy(sl, src)
        # last reader of this tile in the rotation round -> refill it, and
        # bring in the matching second-half ftr jobs on the other queue so
        # queue FIFO order equals consumption order
        if buf == TILE_SZ[t] - 1 and hl + N_BBUF - TILE_SZ[t] + 1 < NH:
            job_lo = hl + N_BBUF - TILE_SZ[t] + 1
            e1 = nc.sync if t % 2 == 0 else nc.scalar
            e2 = nc.scalar if t % 2 == 0 else nc.sync
            scatter(t, job_lo, e1)
            if job_lo >= 24:   # jobs 16-23 ftr already loading on gpsimd
                ftr_chunk(job_lo, job_lo + TILE_SZ[t], e2)
        if hl % OBATCH == OBATCH - 1:
            # outputs ride gpsimd's software DGE queue, keeping both HWDGE
            # queues free for the input stream
            g0 = hl - (OBATCH - 1)
            nc.gpsimd.dma_start(out_ap[:, 2 * g0 : 2 * g0 + 2 * OBATCH, :], ob[:])


def _build_program():
    nc = bacc.Bacc(
        "TRN2", debug=False, enable_asserts=False, target_bir_lowering=False
    )
    ftr_t = nc.dram_tensor("ftr", [CONTR, NH * NT * C], BF16, kind="ExternalInput")
    msk_t = nc.dram_tensor("mskp", [CONTR, NH * JOBW], BF16, kind="ExternalInput")
    out_t = nc.dram_tensor("out", [C, 2 * NH, NB], BF16, kind="ExternalOutput")

    with tile.TileContext(nc) as tc, ExitStack() as ctx:
        _device_body(tc, ctx, out_t.ap(), ftr_t.ap(), msk_t.ap())
    nc.compile()
    return nc


def _prep_ftr(feat_n: np.ndarray, h0: int) -> np.ndarray:
    """[C,H,W] -> ftr[(i,w''), (hl, t, c)] bf16 [100, NH*4*C]:
    ftr[i*20+w'', hl, t, c] = f[c, h0+hl+i-2, 16t+w''-2] (zero-padded)."""
    fT = np.zeros((WP, NROWS, C), ml_dtypes.bfloat16)
    r_lo, r_hi = h0 - 2, h0 + NH + 2
    s_lo, s_hi = max(r_lo, 0), min(r_hi, H)
    fT[PAD : PAD + W, s_lo - r_lo : s_hi - r_lo, :] = (
        feat_n[:, s_lo:s_hi, :].transpose(2, 1, 0).astype(ml_dtypes.bfloat16)
    )
    ftr = np.empty((KS, TPP, NH, NT, C), ml_dtypes.bfloat16)
    for i in range(KS):
        for t in range(NT):
            ftr[i, :, :, t, :] = fT[TP * t : TP * t + TPP, i : i + NH, :]
    return np.ascontiguousarray(ftr.reshape(CONTR, NH * NT * C))


def _prep_mskp(masks_n: np.ndarray, h0: int) -> np.ndarray:
    """[25, 2H, 2W] -> dense band image mskp[(i,w''), (hl, t, col96)] bf16
    [100, NH*384]: run value masks[5i + (4-dw), 2(h0+hl)+a,
    clip(2(16t + w''-4+dw)+b)] at col 4*w'' + (4dw+2b+a); zeros elsewhere.
    """
    t20 = np.arange(SUB)
    dw = t20 // 4
    b = (t20 % 4) // 2
    a = t20 % 2
    i_ar = np.arange(KS).reshape(KS, 1, 1, 1, 1)
    w2 = np.arange(TPP).reshape(1, TPP, 1, 1, 1)
    hl = np.arange(NH).reshape(1, 1, NH, 1, 1)
    tt = np.arange(NT).reshape(1, 1, 1, NT, 1)
    k = 5 * i_ar + (4 - dw)                                  # [5,1,1,1,20]
    hup = 2 * (h0 + hl) + a                                  # [1,1,NH,1,20]
    wup = np.clip(2 * (TP * tt + w2 - 4 + dw) + b, 0, 2 * W - 1)
    vals = masks_n[k, hup, wup].astype(ml_dtypes.bfloat16)   # [5,TPP,NH,NT,20]
    vals = vals.reshape(KS, TPP, NH, NT, KS, 4)              # (.., dw, (b,a))
    band = np.zeros((KS, TPP, NH, NT, BWT), ml_dtypes.bfloat16)
    for w2 in range(TPP):
        for dw in range(KS):
            w = w2 - 4 + dw
            if 0 <= w < TP:
                band[:, w2, :, :, 4 * w : 4 * w + 4] = vals[:, w2, :, :, dw]
    return np.ascontiguousarray(band.reshape(CONTR, NH * JOBW))


def kernel(features: np.ndarray, masks: np.ndarray, _perf: dict | None = None):
    features = np.asarray(features, dtype=np.float32)
    masks = np.asarray(masks, dtype=np.float32)

    if "nc" not in _PROG_CACHE:
        _PROG_CACHE["nc"] = _build_program()
    nc = _PROG_CACHE["nc"]

    in_maps = []
    for core in range(8):
        n, half = divmod(core, 2)
        h0 = NH * half
        in_maps.append(
            {
                "ftr": _prep_ftr(features[n], h0),
                "mskp": _prep_mskp(masks[n], h0),
            }
        )

    trace = bool(_perf is not None and _perf.get("trace"))
    res = run_bass_kernel_spmd(
        nc, in_maps, core_ids=list(range(8)), trace=trace,
        **({} if not trace else {"trace_cores": [0]}),
    )
    if _perf is not None:
        _perf["exec_time_ns"] = res.exec_time_ns
        _perf["trace"] = res.instructions_and_trace

    out = np.empty((N, C, SCALE * H, SCALE * W), np.float32)
    for core in range(8):
        n, half = divmod(core, 2)
        out[n, :, 64 * half : 64 * half + 64, :] = res.results[core]["out"].astype(
            np.float32
        )
    return out
